# revision 11
# baseline (speedup 1.0000x reference)
"""Multi-headed causal attention on 8 trn2 NeuronCores (Bass/Tile).

Sharding: tensor-parallel over heads — 2 heads per core, all 4 batches.
Per core:
  - Q^T/K^T/V^T projections with the 2 heads stacked on the partition axis
    (full 128-wide matmuls, contraction over D streamed from a host-side
    transposed copy of `embedded`).
  - scores computed transposed ([s_k, s_q] layout) so softmax'd tiles feed
    the AV matmul directly as the moving operand; the 2 heads run
    concurrently via PE row-tiling (contraction dim is HD=64 per head).
  - exp on ScalarE (no max subtraction needed: logits are ~N(0,1)); causal
    mask applied by zeroing invalid entries with affine_select on GpSimd.
  - AV matmul uses V augmented with a ones column -> softmax denominators
    come out as row 64 of the context accumulator for free.
  - one 4MB AllToAll redistributes normalized context so each core owns
    1024 rows of (B*S); row-sharded output projection + bias finishes it.
Matmuls run as float32r (full PE rate fp32).
"""
import sys

sys.path.insert(0, "/opt/trn_rl_repo")

import numpy as np

import concourse.bass as bass
import concourse.tile as tile
from concourse import bacc, mybir
from concourse.bass_utils import run_bass_kernel_spmd

B, S, D, H, HD = 4, 2048, 1024, 16, 64
NC_ = 8          # cores
PH = 2           # heads per core
SC = 512         # s_q chunk (psum bank width in fp32)
NK = S // 128    # 16 s_k chunks of 128
ND = D // 128    # 8 contraction chunks of 128
F32 = mybir.dt.float32
F32R = mybir.dt.float32r
EXP = mybir.ActivationFunctionType.Exp
GE = mybir.AluOpType.is_ge

MM_DT = F32R     # matmul compute dtype (bitcast view of f32 tiles)


def _r(ap):
    return ap  # tiles are declared float32r directly


def build():
    nc = bacc.Bacc("TRN2", target_bir_lowering=False, debug=False, num_devices=NC_)

    emb_t = nc.dram_tensor("embedded_t", [B, D, S], F32R, kind="ExternalInput").ap()
    w_qkv = nc.dram_tensor("w_qkv", [3, ND, 128, 128], F32R, kind="ExternalInput").ap()
    wo_t = nc.dram_tensor("wo_t", [ND, 128, D], F32R, kind="ExternalInput").ap()
    bo_row = nc.dram_tensor("bo_row", [1, D], F32, kind="ExternalInput").ap()
    out_shard = nc.dram_tensor("out_shard", [1024, D], F32, kind="ExternalOutput").ap()

    with tile.TileContext(nc) as tc:
        _build_body(nc, tc, emb_t, w_qkv, wo_t, bo_row, out_shard)

    nc.compile()
    return nc


def _build_body(nc, tc, emb_t, w_qkv, wo_t, bo_row, out_shard):
    from contextlib import ExitStack

    ctx = ExitStack()
    with ctx:
        const = ctx.enter_context(tc.tile_pool(name="const", bufs=1))
        ps_mm = ctx.enter_context(tc.tile_pool(name="ps_mm", bufs=4, space="PSUM"))
        ps_ctx = ctx.enter_context(tc.tile_pool(name="ps_ctx", bufs=3, space="PSUM"))
        ps_tr = ctx.enter_context(tc.tile_pool(name="ps_tr", bufs=1, space="PSUM"))
        dram = ctx.enter_context(tc.tile_pool(name="dram", bufs=1, space="DRAM"))

        # ---- phase A: constants ----
        wq_sb = [[const.tile([128, 128], F32R, tag=f"w{p}_{c}", name=f"w{p}_{c}")
                  for c in range(ND)] for p in range(3)]
        for p in range(3):
            for c in range(ND):
                nc.sync.dma_start(out=wq_sb[p][c][:], in_=w_qkv[p, c])

        wot_sb = [const.tile([128, D], F32R, tag=f"wo{c}", name=f"wo{c}")
                  for c in range(ND)]
        for c in range(ND):
            nc.sync.dma_start(out=wot_sb[c][:], in_=wo_t[c])

        bo_sb = const.tile([1, D], F32, tag="bo1")
        nc.sync.dma_start(out=bo_sb[:], in_=bo_row[:])
        bo_b = const.tile([128, D], F32, tag="bob")
        nc.gpsimd.partition_broadcast(bo_b[:], bo_sb[:])

        ones_f32 = const.tile([128, 1], F32, tag="ones_f32")
        nc.vector.memset(ones_f32[:], 1.0)
        ones_r = const.tile([128, 1], F32R, tag="ones_r")
        nc.vector.tensor_copy(ones_r[:], ones_f32[:])

        ident = const.tile([128, 128], F32, tag="ident")
        nc.gpsimd.memset(ident[:], 1.0)
        nc.gpsimd.affine_select(out=ident[:], in_=ident[:], compare_op=GE,
                                fill=0.0, base=0, pattern=[[-1, 128]],
                                channel_multiplier=1)
        nc.gpsimd.affine_select(out=ident[:], in_=ident[:], compare_op=GE,
                                fill=0.0, base=0, pattern=[[1, 128]],
                                channel_multiplier=-1)

        a2a_in = dram.tile([NC_, 128, 1024], F32R, tag="a2a_in")
        a2a_out = dram.tile([NC_, 128, 1024], F32R, tag="a2a_out")

        # ---- per-batch: projections + attention ----
        attn_ctx = ExitStack()
        etp = attn_ctx.enter_context(tc.tile_pool(name="etp", bufs=12))
        qtp = attn_ctx.enter_context(tc.tile_pool(name="qtp", bufs=2))
        ktp = attn_ctx.enter_context(tc.tile_pool(name="ktp", bufs=2))
        vtp = attn_ctx.enter_context(tc.tile_pool(name="vtp", bufs=2))
        vsb = attn_ctx.enter_context(tc.tile_pool(name="vsb", bufs=2))
        exp_p = attn_ctx.enter_context(tc.tile_pool(name="exp_p", bufs=6))
        cn_p = attn_ctx.enter_context(tc.tile_pool(name="cn_p", bufs=3))
        rc_p = attn_ctx.enter_context(tc.tile_pool(name="rc_p", bufs=2))
        rb_p = attn_ctx.enter_context(tc.tile_pool(name="rb_p", bufs=2))
        for b in range(B):
            # E^T tiles: [128 d, 1024 s] halves (contiguous partition lines)
            et = {}
            for half in range(2):
                for c in range(ND):
                    t = etp.tile([128, 1024], F32R, tag="et")
                    nc.sync.dma_start(
                        out=t[:], in_=emb_t[b, 128 * c:128 * (c + 1),
                                            1024 * half:1024 * (half + 1)])
                    et[(half, c)] = t

            qt = qtp.tile([128, S], F32R, tag="qt")
            kt = ktp.tile([128, S], F32R, tag="kt")
            vt = vtp.tile([128, S], F32, tag="vt")
            dests = [qt, kt, vt]
            for j4 in range(4):          # s chunks of 512
                half, off = j4 // 2, (j4 % 2) * SC
                for p in range(3):
                    ps = ps_mm.tile([128, SC], F32, tag="mm")
                    for c in range(ND):
                        nc.tensor.matmul(
                            ps[:], lhsT=_r(wq_sb[p][c][:]),
                            rhs=_r(et[(half, c)][:, off:off + SC]),
                            start=(c == 0), stop=(c == ND - 1))
                    nc.vector.tensor_copy(dests[p][:, SC * j4:SC * (j4 + 1)], ps[:])

            # V natural layout (+ ones column for softmax denominators)
            v01 = [vsb.tile([128, NK, 65], F32R, tag=f"v{h}", name=f"v{h}")
                   for h in range(PH)]
            for sk in range(NK):
                pt = ps_tr.tile([128, 128], F32, tag="tr")
                nc.tensor.transpose(pt[:], vt[:, 128 * sk:128 * (sk + 1)], ident[:])
                for h in range(PH):
                    nc.vector.tensor_copy(v01[h][:, sk, 0:64],
                                          pt[:, 64 * h:64 * (h + 1)])
                    nc.vector.tensor_copy(v01[h][:, sk, 64:65], ones_r[:])

            # attention, one s_q chunk of 512 at a time
            for j in range(4):
                mtop = 4 * j + 4
                ctx_ps = [ps_ctx.tile([65, SC], F32, tag="ctx", name=f"ctx{b}_{j}_{h}")
                          for h in range(PH)]
                pend, pend_m = None, -1
                for m in range(mtop):
                    cur = []
                    for h in range(PH):
                        sc_ps = ps_mm.tile([128, SC], F32, tag="mm")
                        nc.tensor.matmul(
                            sc_ps[:],
                            lhsT=_r(kt[64 * h:64 * (h + 1), 128 * m:128 * (m + 1)]),
                            rhs=_r(qt[64 * h:64 * (h + 1), SC * j:SC * (j + 1)]),
                            start=True, stop=True, tile_position=(64 * h, 0))
                        ex = exp_p.tile([128, SC], F32R, tag="ex")
                        nc.scalar.activation(out=ex[:], in_=sc_ps[:], func=EXP,
                                             scale=0.125)
                        if m >= 4 * j:  # diagonal tile: zero k>q entries
                            nc.gpsimd.affine_select(
                                out=ex[:], in_=ex[:], compare_op=GE, fill=0.0,
                                base=SC * j - 128 * m, pattern=[[1, SC]],
                                channel_multiplier=-1)
                        cur.append(ex)
                    if pend is not None:
                        for h in range(PH):
                            nc.tensor.matmul(
                                ctx_ps[h][:], lhsT=_r(v01[h][:, pend_m, :]),
                                rhs=_r(pend[h][:]),
                                start=(pend_m == 0), stop=False)
                    pend, pend_m = cur, m
                for h in range(PH):
                    nc.tensor.matmul(
                        ctx_ps[h][:], lhsT=_r(v01[h][:, pend_m, :]),
                        rhs=_r(pend[h][:]),
                        start=(pend_m == 0), stop=True)
                # normalize by the ones-row denominator, stage for all-to-all
                o, col = 2 * b + j // 2, SC * (j % 2)
                for h in range(PH):
                    rc = rc_p.tile([1, SC], F32, tag="rc")
                    nc.vector.reciprocal(rc[:], ctx_ps[h][64:65, :])
                    rb = rb_p.tile([64, SC], F32, tag="rb")
                    nc.gpsimd.partition_broadcast(rb[:], rc[:])
                    cn = cn_p.tile([64, SC], F32R, tag="cn")
                    nc.vector.tensor_mul(cn[:], ctx_ps[h][0:64, :], rb[:])
                    nc.sync.dma_start(
                        out=a2a_in[o, 64 * h:64 * (h + 1), col:col + SC],
                        in_=cn[:])

        # ---- all-to-all + row-sharded output projection ----
        attn_ctx.close()
        nc.gpsimd.collective_compute(
            "AllToAll", mybir.AluOpType.bypass,
            replica_groups=[list(range(NC_))],
            ins=[a2a_in.opt()], outs=[a2a_out.opt()])

        cat_p = ctx.enter_context(tc.tile_pool(name="cat_p", bufs=8))
        ob_p = ctx.enter_context(tc.tile_pool(name="ob_p", bufs=3))
        cats = []
        for r in range(NC_):
            ct = cat_p.tile([128, 1024], F32R, tag="cat")
            nc.sync.dma_start(out=ct[:], in_=a2a_out[r])
            cats.append(ct)
        for sq in range(8):
            for n in range(2):
                po = ps_mm.tile([128, SC], F32, tag="mm")
                for kp in range(ND):
                    nc.tensor.matmul(
                        po[:], lhsT=_r(cats[kp][:, 128 * sq:128 * (sq + 1)]),
                        rhs=_r(wot_sb[kp][:, SC * n:SC * (n + 1)]),
                        start=(kp == 0), stop=(kp == ND - 1))
                ob = ob_p.tile([128, SC], F32, tag="ob")
                nc.vector.tensor_add(ob[:], po[:], bo_b[:, SC * n:SC * (n + 1)])
                nc.sync.dma_start(
                    out=out_shard[128 * sq:128 * (sq + 1), SC * n:SC * (n + 1)],
                    in_=ob[:])


_NC_CACHE = None


def _get_nc():
    global _NC_CACHE
    if _NC_CACHE is None:
        _NC_CACHE = build()
    return _NC_CACHE


def _round_fp32r(x):
    """Round fp32 to fp32r (11-bit mantissa, RNE) — what the PE expects."""
    u = np.ascontiguousarray(x, np.float32).view(np.uint32)
    r = (u + np.uint32(0x7FF) + ((u >> np.uint32(12)) & np.uint32(1))) & np.uint32(0xFFFFF000)
    return r.view(np.float32)


def kernel(embedded, Wq, Wk, Wv, Wo, bo, _trace=False):
    embedded = np.ascontiguousarray(np.asarray(embedded, np.float32))
    emb_t = _round_fp32r(np.ascontiguousarray(embedded.transpose(0, 2, 1)))
    W = _round_fp32r(np.stack([np.asarray(Wq), np.asarray(Wk), np.asarray(Wv)]).astype(np.float32))
    wo_t = _round_fp32r(np.ascontiguousarray(np.asarray(Wo, np.float32).T)).reshape(ND, 128, D)
    bo_row = np.asarray(bo, np.float32).reshape(1, D)

    in_maps = []
    for c in range(NC_):
        w = W[:, 2 * c:2 * c + 2]                  # [3, 2, D, HD]
        w = np.ascontiguousarray(w.transpose(0, 2, 1, 3)).reshape(3, ND, 128, 128)
        in_maps.append({
            "embedded_t": emb_t,
            "w_qkv": w,
            "wo_t": wo_t,
            "bo_row": bo_row,
        })

    nc = _get_nc()
    res = run_bass_kernel_spmd(nc, in_maps, core_ids=list(range(NC_)),
                               trace=_trace)

    out = np.empty((B, S, D), np.float32)
    for c in range(NC_):
        s0 = (c % 2) * 1024
        out[c // 2, s0:s0 + 1024, :] = res.results[c]["out_shard"]
    if _trace:
        return out, res
    return out


# revision 13
# speedup vs baseline: 1.1096x; 1.1096x over previous
"""Multi-headed causal attention on 8 trn2 NeuronCores (Bass/Tile).

Sharding: tensor-parallel over heads — 2 heads per core, all 4 batches.
Per core:
  - Q^T/K^T/V^T projections with the 2 heads stacked on the partition axis
    (full 128-wide matmuls, contraction over D streamed from a host-side
    transposed copy of `embedded`).
  - scores computed transposed ([s_k, s_q] layout) so softmax'd tiles feed
    the AV matmul directly as the moving operand; the 2 heads run
    concurrently via PE row-tiling (contraction dim is HD=64 per head).
  - exp on ScalarE (no max subtraction needed: logits are ~N(0,1)); causal
    mask applied by zeroing invalid entries with affine_select on GpSimd.
  - AV matmul uses V augmented with a ones column -> softmax denominators
    come out as row 64 of the context accumulator for free.
  - one 4MB AllToAll redistributes normalized context so each core owns
    1024 rows of (B*S); row-sharded output projection + bias finishes it.
Matmuls run as float32r (full PE rate fp32).
"""
import sys

sys.path.insert(0, "/opt/trn_rl_repo")

import numpy as np

import concourse.bass as bass
import concourse.tile as tile
from concourse import bacc, mybir
from concourse.bass_utils import run_bass_kernel_spmd

B, S, D, H, HD = 4, 2048, 1024, 16, 64
NC_ = 8          # cores
PH = 2           # heads per core
SC = 512         # s_q chunk (psum bank width in fp32)
NK = S // 128    # 16 s_k chunks of 128
ND = D // 128    # 8 contraction chunks of 128
F32 = mybir.dt.float32
F32R = mybir.dt.float32r
BF16 = mybir.dt.bfloat16
EXP = mybir.ActivationFunctionType.Exp
GE = mybir.AluOpType.is_ge

MM_DT = F32R     # matmul compute dtype (bitcast view of f32 tiles)


def _r(ap):
    return ap  # tiles are declared float32r directly


def build():
    nc = bacc.Bacc("TRN2", target_bir_lowering=False, debug=False, num_devices=NC_)

    emb_t = nc.dram_tensor("embedded_t", [B, D, S], F32R, kind="ExternalInput").ap()
    w_qkv = nc.dram_tensor("w_qkv", [3, ND, 128, 128], F32R, kind="ExternalInput").ap()
    wo_t = nc.dram_tensor("wo_t", [ND, 128, D], F32R, kind="ExternalInput").ap()
    bo_row = nc.dram_tensor("bo_row", [1, D], F32, kind="ExternalInput").ap()
    out_shard = nc.dram_tensor("out_shard", [1024, D], F32, kind="ExternalOutput").ap()

    with tile.TileContext(nc) as tc:
        _build_body(nc, tc, emb_t, w_qkv, wo_t, bo_row, out_shard)

    nc.compile()
    return nc


def _build_body(nc, tc, emb_t, w_qkv, wo_t, bo_row, out_shard):
    from contextlib import ExitStack

    ctx = ExitStack()
    with ctx:
        const = ctx.enter_context(tc.tile_pool(name="const", bufs=1))
        ps_mm = ctx.enter_context(tc.tile_pool(name="ps_mm", bufs=6, space="PSUM"))
        ps_ctx = ctx.enter_context(tc.tile_pool(name="ps_ctx", bufs=2, space="PSUM"))
        dram = ctx.enter_context(tc.tile_pool(name="dram", bufs=1, space="DRAM"))

        # ---- phase A: constants ----
        wq_sb = [[const.tile([128, 128], F32R, tag=f"w{p}_{c}", name=f"w{p}_{c}")
                  for c in range(ND)] for p in range(3)]
        for p in range(3):
            for c in range(ND):
                nc.sync.dma_start(out=wq_sb[p][c][:], in_=w_qkv[p, c])

        wot_sb = [const.tile([128, D], F32R, tag=f"wo{c}", name=f"wo{c}")
                  for c in range(ND)]
        for c in range(ND):
            nc.sync.dma_start(out=wot_sb[c][:], in_=wo_t[c])

        bo_sb = const.tile([1, D], F32, tag="bo1")
        nc.sync.dma_start(out=bo_sb[:], in_=bo_row[:])
        bo_b = const.tile([128, D], F32, tag="bob")
        nc.gpsimd.partition_broadcast(bo_b[:], bo_sb[:])

        ones_f32 = const.tile([128, 1], F32, tag="ones_f32")
        nc.vector.memset(ones_f32[:], 1.0)
        ones_r = const.tile([128, 1], BF16, tag="ones_r")
        nc.vector.tensor_copy(ones_r[:], ones_f32[:])

        ident = const.tile([128, 128], F32, tag="ident")
        nc.gpsimd.memset(ident[:], 1.0)
        nc.gpsimd.affine_select(out=ident[:], in_=ident[:], compare_op=GE,
                                fill=0.0, base=0, pattern=[[-1, 128]],
                                channel_multiplier=1)
        nc.gpsimd.affine_select(out=ident[:], in_=ident[:], compare_op=GE,
                                fill=0.0, base=0, pattern=[[1, 128]],
                                channel_multiplier=-1)

        a2a_in = dram.tile([NC_, 128, 1024], F32R, tag="a2a_in")
        a2a_out = dram.tile([NC_, 128, 1024], F32R, tag="a2a_out")

        # ---- per-batch: projections + attention ----
        attn_ctx = ExitStack()
        etp = attn_ctx.enter_context(tc.tile_pool(name="etp", bufs=12))
        qtp = attn_ctx.enter_context(tc.tile_pool(name="qtp", bufs=2))
        ktp = attn_ctx.enter_context(tc.tile_pool(name="ktp", bufs=2))
        vtp = attn_ctx.enter_context(tc.tile_pool(name="vtp", bufs=2))
        vsb = attn_ctx.enter_context(tc.tile_pool(name="vsb", bufs=2))
        exp_p = attn_ctx.enter_context(tc.tile_pool(name="exp_p", bufs=6))
        cn_p = attn_ctx.enter_context(tc.tile_pool(name="cn_p", bufs=3))
        rc_p = attn_ctx.enter_context(tc.tile_pool(name="rc_p", bufs=2))
        rb_p = attn_ctx.enter_context(tc.tile_pool(name="rb_p", bufs=2))
        for b in range(B):
            # E^T tiles: [128 d, 1024 s] halves (contiguous partition lines)
            et = {}
            for half in range(2):
                for c in range(ND):
                    t = etp.tile([128, 1024], F32R, tag="et")
                    nc.sync.dma_start(
                        out=t[:], in_=emb_t[b, 128 * c:128 * (c + 1),
                                            1024 * half:1024 * (half + 1)])
                    et[(half, c)] = t

            qt = qtp.tile([128, S], BF16, tag="qt")
            kt = ktp.tile([128, S], BF16, tag="kt")
            vt = vtp.tile([128, S], F32, tag="vt")
            dests = [qt, kt, vt]
            for j4 in range(4):          # s chunks of 512
                half, off = j4 // 2, (j4 % 2) * SC
                for p in range(3):
                    ps = ps_mm.tile([128, SC], F32, tag="mm")
                    for c in range(ND):
                        nc.tensor.matmul(
                            ps[:], lhsT=_r(wq_sb[p][c][:]),
                            rhs=_r(et[(half, c)][:, off:off + SC]),
                            start=(c == 0), stop=(c == ND - 1))
                    nc.vector.tensor_copy(dests[p][:, SC * j4:SC * (j4 + 1)], ps[:])

            # V natural layout (+ ones column for softmax denominators)
            v01 = [vsb.tile([128, NK, 65], BF16, tag=f"v{h}", name=f"v{h}")
                   for h in range(PH)]
            for sk in range(NK):
                pt = ps_mm.tile([128, 128], F32, tag="mm", name=f"tr{b}_{sk}")
                nc.tensor.transpose(pt[:], vt[:, 128 * sk:128 * (sk + 1)], ident[:])
                for h in range(PH):
                    nc.vector.tensor_copy(v01[h][:, sk, 0:64],
                                          pt[:, 64 * h:64 * (h + 1)])
                    nc.vector.tensor_copy(v01[h][:, sk, 64:65], ones_r[:])

            # attention, one s_q chunk of 512 at a time
            for j in range(4):
                mtop = 4 * j + 4
                ctx_ps = [ps_ctx.tile([65, SC], F32, tag="ctx", name=f"ctx{b}_{j}_{h}")
                          for h in range(PH)]
                PIPE = 2   # scores run this many m-iterations ahead of AV
                exq = []   # (m, [ex_h0, ex_h1]) awaiting AV

                def emit_scores(m):
                    cur = []
                    for h in range(PH):
                        sc_ps = ps_mm.tile([128, SC], F32, tag="mm",
                                           name=f"sc{b}_{j}_{m}_{h}")
                        nc.tensor.matmul(
                            sc_ps[:],
                            lhsT=kt[64 * h:64 * (h + 1), 128 * m:128 * (m + 1)],
                            rhs=qt[64 * h:64 * (h + 1), SC * j:SC * (j + 1)],
                            start=True, stop=True, tile_position=(64 * h, 0))
                        ex = exp_p.tile([128, SC], BF16, tag="ex",
                                        name=f"ex{b}_{j}_{m}_{h}")
                        nc.scalar.activation(out=ex[:], in_=sc_ps[:], func=EXP,
                                             scale=0.125)
                        if m >= 4 * j:  # diagonal tile: zero k>q entries
                            nc.gpsimd.affine_select(
                                out=ex[:], in_=ex[:], compare_op=GE, fill=0.0,
                                base=SC * j - 128 * m, pattern=[[1, SC]],
                                channel_multiplier=-1)
                        cur.append(ex)
                    exq.append((m, cur))

                def emit_av():
                    m_av, tiles = exq.pop(0)
                    for h in range(PH):
                        nc.tensor.matmul(
                            ctx_ps[h][:], lhsT=v01[h][:, m_av, :],
                            rhs=tiles[h][:],
                            start=(m_av == 0), stop=(m_av == mtop - 1))

                for m in range(mtop):
                    emit_scores(m)
                    if len(exq) > PIPE:
                        emit_av()
                while exq:
                    emit_av()
                # normalize by the ones-row denominator, stage for all-to-all
                o, col = 2 * b + j // 2, SC * (j % 2)
                for h in range(PH):
                    rc = rc_p.tile([1, SC], F32, tag="rc")
                    nc.vector.reciprocal(rc[:], ctx_ps[h][64:65, :])
                    rb = rb_p.tile([64, SC], F32, tag="rb")
                    nc.gpsimd.partition_broadcast(rb[:], rc[:])
                    cn = cn_p.tile([64, SC], F32R, tag="cn")
                    nc.vector.tensor_mul(cn[:], ctx_ps[h][0:64, :], rb[:])
                    nc.sync.dma_start(
                        out=a2a_in[o, 64 * h:64 * (h + 1), col:col + SC],
                        in_=cn[:])

        # ---- all-to-all + row-sharded output projection ----
        attn_ctx.close()
        nc.gpsimd.collective_compute(
            "AllToAll", mybir.AluOpType.bypass,
            replica_groups=[list(range(NC_))],
            ins=[a2a_in.opt()], outs=[a2a_out.opt()])

        cat_p = ctx.enter_context(tc.tile_pool(name="cat_p", bufs=8))
        ob_p = ctx.enter_context(tc.tile_pool(name="ob_p", bufs=3))
        cats = []
        for r in range(NC_):
            ct = cat_p.tile([128, 1024], F32R, tag="cat")
            nc.sync.dma_start(out=ct[:], in_=a2a_out[r])
            cats.append(ct)
        for sq in range(8):
            for n in range(2):
                po = ps_mm.tile([128, SC], F32, tag="mm")
                for kp in range(ND):
                    nc.tensor.matmul(
                        po[:], lhsT=_r(cats[kp][:, 128 * sq:128 * (sq + 1)]),
                        rhs=_r(wot_sb[kp][:, SC * n:SC * (n + 1)]),
                        start=(kp == 0), stop=(kp == ND - 1))
                ob = ob_p.tile([128, SC], F32, tag="ob")
                nc.vector.tensor_add(ob[:], po[:], bo_b[:, SC * n:SC * (n + 1)])
                nc.sync.dma_start(
                    out=out_shard[128 * sq:128 * (sq + 1), SC * n:SC * (n + 1)],
                    in_=ob[:])


_NC_CACHE = None


def _get_nc():
    global _NC_CACHE
    if _NC_CACHE is None:
        _NC_CACHE = build()
    return _NC_CACHE


def _round_fp32r(x):
    """Round fp32 to fp32r (11-bit mantissa, RNE) — what the PE expects."""
    u = np.ascontiguousarray(x, np.float32).view(np.uint32)
    r = (u + np.uint32(0x7FF) + ((u >> np.uint32(12)) & np.uint32(1))) & np.uint32(0xFFFFF000)
    return r.view(np.float32)


def kernel(embedded, Wq, Wk, Wv, Wo, bo, _trace=False):
    embedded = np.ascontiguousarray(np.asarray(embedded, np.float32))
    emb_t = _round_fp32r(np.ascontiguousarray(embedded.transpose(0, 2, 1)))
    W = _round_fp32r(np.stack([np.asarray(Wq), np.asarray(Wk), np.asarray(Wv)]).astype(np.float32))
    wo_t = _round_fp32r(np.ascontiguousarray(np.asarray(Wo, np.float32).T)).reshape(ND, 128, D)
    bo_row = np.asarray(bo, np.float32).reshape(1, D)

    in_maps = []
    for c in range(NC_):
        w = W[:, 2 * c:2 * c + 2]                  # [3, 2, D, HD]
        w = np.ascontiguousarray(w.transpose(0, 2, 1, 3)).reshape(3, ND, 128, 128)
        in_maps.append({
            "embedded_t": emb_t,
            "w_qkv": w,
            "wo_t": wo_t,
            "bo_row": bo_row,
        })

    nc = _get_nc()
    res = run_bass_kernel_spmd(nc, in_maps, core_ids=list(range(NC_)),
                               trace=_trace)

    out = np.empty((B, S, D), np.float32)
    for c in range(NC_):
        s0 = (c % 2) * 1024
        out[c // 2, s0:s0 + 1024, :] = res.results[c]["out_shard"]
    if _trace:
        return out, res
    return out


# revision 14
# speedup vs baseline: 1.3946x; 1.2568x over previous
"""Multi-headed causal attention on 8 trn2 NeuronCores (Bass/Tile).

Sharding: tensor-parallel over heads — 2 heads per core, all 4 batches.
Per core:
  - Q^T/K^T/V^T projections with the 2 heads stacked on the partition axis
    (full 128-wide fp32r matmuls, contraction over D streamed from a
    host-side transposed copy of `embedded`).
  - scores computed transposed ([s_k, s_q] layout) in bf16 with K padded
    to 128 by zero rows (per-head K tiles carry zeros in the other head's
    partition range, so the stacked Q^T works as the moving operand for
    both heads and every matmul is the fast full-array shape).
  - both heads' score tiles land in one 2-bank PSUM tile; a single exp on
    ScalarE (no max subtraction needed: logits are ~N(0,1)); causal mask
    zeroes invalid entries via one affine_select on GpSimd.
  - AV matmul uses V padded to 128 columns: 64 V cols + a ones column
    (softmax denominators fall out as row 64) + zeros.
  - context is copied out of PSUM immediately (frees banks), normalized
    off the critical path, and staged for one 4MB AllToAll; row-sharded
    fp32r output projection + bias finishes it.
"""
import sys

sys.path.insert(0, "/opt/trn_rl_repo")

import numpy as np

import concourse.bass as bass
import concourse.tile as tile
from concourse import bacc, mybir
from concourse.bass_utils import run_bass_kernel_spmd

B, S, D, H, HD = 4, 2048, 1024, 16, 64
NC_ = 8          # cores
PH = 2           # heads per core
SC = 512         # s_q chunk (psum bank width in fp32)
NK = S // 128    # 16 s_k chunks of 128
ND = D // 128    # 8 contraction chunks of 128
F32 = mybir.dt.float32
F32R = mybir.dt.float32r
BF16 = mybir.dt.bfloat16
EXP = mybir.ActivationFunctionType.Exp
GE = mybir.AluOpType.is_ge


def build():
    nc = bacc.Bacc("TRN2", target_bir_lowering=False, debug=False, num_devices=NC_)

    emb_t = nc.dram_tensor("embedded_t", [B, D, S], F32R, kind="ExternalInput").ap()
    w_qkv = nc.dram_tensor("w_qkv", [3, ND, 128, 128], F32R, kind="ExternalInput").ap()
    wo_t = nc.dram_tensor("wo_t", [ND, 128, D], F32R, kind="ExternalInput").ap()
    bo_row = nc.dram_tensor("bo_row", [1, D], F32, kind="ExternalInput").ap()
    out_shard = nc.dram_tensor("out_shard", [1024, D], F32, kind="ExternalOutput").ap()

    with tile.TileContext(nc) as tc:
        _build_body(nc, tc, emb_t, w_qkv, wo_t, bo_row, out_shard)

    nc.compile()
    return nc


def _build_body(nc, tc, emb_t, w_qkv, wo_t, bo_row, out_shard):
    from contextlib import ExitStack

    ctx = ExitStack()
    with ctx:
        const = ctx.enter_context(tc.tile_pool(name="const", bufs=1))
        # "mm" slots are sized [128, 1024] (2 PSUM banks): 3x2 + ctx 2x1 = 8
        ps_mm = ctx.enter_context(tc.tile_pool(name="ps_mm", bufs=3, space="PSUM"))
        ps_ctx = ctx.enter_context(tc.tile_pool(name="ps_ctx", bufs=2, space="PSUM"))
        dram = ctx.enter_context(tc.tile_pool(name="dram", bufs=1, space="DRAM"))

        attn_ctx = ExitStack()
        etp = attn_ctx.enter_context(tc.tile_pool(name="etp", bufs=12))
        qtp = attn_ctx.enter_context(tc.tile_pool(name="qtp", bufs=2))
        ktp = attn_ctx.enter_context(tc.tile_pool(name="ktp", bufs=2))
        vtp = attn_ctx.enter_context(tc.tile_pool(name="vtp", bufs=2))
        vsb = attn_ctx.enter_context(tc.tile_pool(name="vsb", bufs=2))
        exp_p = attn_ctx.enter_context(tc.tile_pool(name="exp_p", bufs=5))
        cu_p = attn_ctx.enter_context(tc.tile_pool(name="cu_p", bufs=3))
        cn_p = attn_ctx.enter_context(tc.tile_pool(name="cn_p", bufs=3))
        rc_p = attn_ctx.enter_context(tc.tile_pool(name="rc_p", bufs=2))
        rb_p = attn_ctx.enter_context(tc.tile_pool(name="rb_p", bufs=2))

        # ---- prefetch batch 0 activations before anything else ----
        et_pre = {}
        for half in range(2):
            for c in range(ND):
                t = etp.tile([128, 1024], F32R, tag="et", name=f"et0_{half}_{c}")
                nc.sync.dma_start(
                    out=t[:], in_=emb_t[0, 128 * c:128 * (c + 1),
                                        1024 * half:1024 * (half + 1)])
                et_pre[(half, c)] = t

        # ---- constants (wo_t loads happen in the output phase) ----
        wq_sb = [[const.tile([128, 128], F32R, tag=f"w{p}_{c}", name=f"w{p}_{c}")
                  for c in range(ND)] for p in range(3)]
        for p in range(3):
            for c in range(ND):
                nc.sync.dma_start(out=wq_sb[p][c][:], in_=w_qkv[p, c])

        bo_sb = const.tile([1, D], F32, tag="bo1")
        nc.sync.dma_start(out=bo_sb[:], in_=bo_row[:])
        bo_b = const.tile([128, D], F32, tag="bob")
        nc.gpsimd.partition_broadcast(bo_b[:], bo_sb[:])

        ones_f32 = const.tile([128, 1], F32, tag="ones_f32")
        nc.vector.memset(ones_f32[:], 1.0)
        ones_r = const.tile([128, 1], BF16, tag="ones_r")
        nc.vector.tensor_copy(ones_r[:], ones_f32[:])

        ident = const.tile([128, 128], F32, tag="ident")
        nc.gpsimd.memset(ident[:], 1.0)
        nc.gpsimd.affine_select(out=ident[:], in_=ident[:], compare_op=GE,
                                fill=0.0, base=0, pattern=[[-1, 128]],
                                channel_multiplier=1)
        nc.gpsimd.affine_select(out=ident[:], in_=ident[:], compare_op=GE,
                                fill=0.0, base=0, pattern=[[1, 128]],
                                channel_multiplier=-1)

        a2a_in = dram.tile([NC_, 128, 1024], F32R, tag="a2a_in")
        a2a_out = dram.tile([NC_, 128, 1024], F32R, tag="a2a_out")

        # ---- per-batch: projections + attention ----
        for b in range(B):
            if b == 0:
                et = et_pre
            else:
                et = {}
                for half in range(2):
                    for c in range(ND):
                        t = etp.tile([128, 1024], F32R, tag="et",
                                     name=f"et{b}_{half}_{c}")
                        nc.sync.dma_start(
                            out=t[:], in_=emb_t[b, 128 * c:128 * (c + 1),
                                                1024 * half:1024 * (half + 1)])
                        et[(half, c)] = t

            qt = qtp.tile([128, S], BF16, tag="qt")
            # per-head K^T padded to K=128 with zero rows for the other head
            kt0 = ktp.tile([128, S], BF16, tag="kt0")
            kt1 = ktp.tile([128, S], BF16, tag="kt1")
            nc.vector.memset(kt0[64:128, :], 0.0)
            nc.vector.memset(kt1[0:64, :], 0.0)
            vt = vtp.tile([128, S], F32, tag="vt")
            for j4 in range(4):          # s chunks of 512
                half, off = j4 // 2, (j4 % 2) * SC
                sl = slice(SC * j4, SC * (j4 + 1))
                for p in range(3):
                    ps = ps_mm.tile([128, SC], F32, tag="mm",
                                    name=f"pj{b}_{j4}_{p}")
                    for c in range(ND):
                        nc.tensor.matmul(
                            ps[:], lhsT=wq_sb[p][c][:],
                            rhs=et[(half, c)][:, off:off + SC],
                            start=(c == 0), stop=(c == ND - 1))
                    if p == 0:
                        nc.vector.tensor_copy(qt[:, sl], ps[:])
                    elif p == 1:
                        nc.vector.tensor_copy(kt0[0:64, sl], ps[0:64, :])
                        nc.vector.tensor_copy(kt1[64:128, sl], ps[64:128, :])
                    else:
                        nc.vector.tensor_copy(vt[:, sl], ps[:])
            kts = [kt0, kt1]

            # V natural layout padded to 128 cols: V | ones | zeros
            v01 = [vsb.tile([128, NK, 128], BF16, tag=f"v{h}", name=f"v{h}")
                   for h in range(PH)]
            for h in range(PH):
                nc.vector.memset(v01[h][:, :, 65:128], 0.0)
            for sk in range(NK):
                pt = ps_mm.tile([128, 128], F32, tag="mm", name=f"tr{b}_{sk}")
                nc.tensor.transpose(pt[:], vt[:, 128 * sk:128 * (sk + 1)], ident[:])
                for h in range(PH):
                    nc.vector.tensor_copy(v01[h][:, sk, 0:64],
                                          pt[:, 64 * h:64 * (h + 1)])
                    nc.vector.tensor_copy(v01[h][:, sk, 64:65], ones_r[:])

            # attention, one s_q chunk of 512 at a time
            for j in range(4):
                mtop = 4 * j + 4
                ctx_ps = [ps_ctx.tile([128, SC], F32, tag="ctx",
                                      name=f"ctx{b}_{j}_{h}")
                          for h in range(PH)]
                PIPE = 2   # scores run this many m-iterations ahead of AV
                exq = []   # (m, ex) awaiting AV

                def emit_scores(m):
                    psc = ps_mm.tile([128, 2 * SC], F32, tag="mm",
                                     name=f"sc{b}_{j}_{m}")
                    for h in range(PH):
                        nc.tensor.matmul(
                            psc[:, SC * h:SC * (h + 1)],
                            lhsT=kts[h][:, 128 * m:128 * (m + 1)],
                            rhs=qt[:, SC * j:SC * (j + 1)],
                            start=True, stop=True)
                    ex = exp_p.tile([128, 2 * SC], BF16, tag="ex",
                                    name=f"ex{b}_{j}_{m}")
                    nc.scalar.activation(out=ex[:], in_=psc[:], func=EXP,
                                         scale=0.125)
                    if m >= 4 * j:  # diagonal tile: zero k>q entries
                        nc.gpsimd.affine_select(
                            out=ex[:], in_=ex[:], compare_op=GE, fill=0.0,
                            base=SC * j - 128 * m, pattern=[[0, 2], [1, SC]],
                            channel_multiplier=-1)
                    exq.append((m, ex))

                def emit_av():
                    m_av, ex = exq.pop(0)
                    for h in range(PH):
                        nc.tensor.matmul(
                            ctx_ps[h][:], lhsT=v01[h][:, m_av, :],
                            rhs=ex[:, SC * h:SC * (h + 1)],
                            start=(m_av == 0), stop=(m_av == mtop - 1))

                for m in range(mtop):
                    emit_scores(m)
                    if len(exq) > PIPE:
                        emit_av()
                while exq:
                    emit_av()

                # free PSUM fast, then normalize off the critical path
                o, col = 2 * b + j // 2, SC * (j % 2)
                for h in range(PH):
                    cu = cu_p.tile([65, SC], F32, tag="cu",
                                   name=f"cu{b}_{j}_{h}")
                    nc.vector.tensor_copy(cu[:], ctx_ps[h][0:65, :])
                    rc = rc_p.tile([1, SC], F32, tag="rc")
                    nc.vector.reciprocal(rc[:], cu[64:65, :])
                    rb = rb_p.tile([64, SC], F32, tag="rb")
                    nc.gpsimd.partition_broadcast(rb[:], rc[:])
                    cn = cn_p.tile([64, SC], F32R, tag="cn")
                    nc.vector.tensor_mul(cn[:], cu[0:64, :], rb[:])
                    nc.sync.dma_start(
                        out=a2a_in[o, 64 * h:64 * (h + 1), col:col + SC],
                        in_=cn[:])

        # ---- all-to-all + row-sharded output projection ----
        attn_ctx.close()
        nc.gpsimd.collective_compute(
            "AllToAll", mybir.AluOpType.bypass,
            replica_groups=[list(range(NC_))],
            ins=[a2a_in.opt()], outs=[a2a_out.opt()])

        cat_p = ctx.enter_context(tc.tile_pool(name="cat_p", bufs=8))
        ob_p = ctx.enter_context(tc.tile_pool(name="ob_p", bufs=3))
        wot_sb = [const.tile([128, D], F32R, tag=f"wo{c}", name=f"wo{c}")
                  for c in range(ND)]
        for c in range(ND):
            nc.sync.dma_start(out=wot_sb[c][:], in_=wo_t[c])
        cats = []
        for r in range(NC_):
            ct = cat_p.tile([128, 1024], F32R, tag="cat", name=f"cat{r}")
            nc.sync.dma_start(out=ct[:], in_=a2a_out[r])
            cats.append(ct)
        for sq in range(8):
            for n in range(2):
                po = ps_mm.tile([128, SC], F32, tag="mm", name=f"po{sq}_{n}")
                for kp in range(ND):
                    nc.tensor.matmul(
                        po[:], lhsT=cats[kp][:, 128 * sq:128 * (sq + 1)],
                        rhs=wot_sb[kp][:, SC * n:SC * (n + 1)],
                        start=(kp == 0), stop=(kp == ND - 1))
                ob = ob_p.tile([128, SC], F32, tag="ob")
                nc.vector.tensor_add(ob[:], po[:], bo_b[:, SC * n:SC * (n + 1)])
                nc.sync.dma_start(
                    out=out_shard[128 * sq:128 * (sq + 1), SC * n:SC * (n + 1)],
                    in_=ob[:])


_NC_CACHE = None


def _get_nc():
    global _NC_CACHE
    if _NC_CACHE is None:
        _NC_CACHE = build()
    return _NC_CACHE


def _round_fp32r(x):
    """Round fp32 to fp32r (11-bit mantissa, RNE) — what the PE expects."""
    u = np.ascontiguousarray(x, np.float32).view(np.uint32)
    r = (u + np.uint32(0x7FF) + ((u >> np.uint32(12)) & np.uint32(1))) & np.uint32(0xFFFFF000)
    return r.view(np.float32)


def kernel(embedded, Wq, Wk, Wv, Wo, bo, _trace=False):
    embedded = np.ascontiguousarray(np.asarray(embedded, np.float32))
    emb_t = _round_fp32r(np.ascontiguousarray(embedded.transpose(0, 2, 1)))
    W = _round_fp32r(np.stack([np.asarray(Wq), np.asarray(Wk), np.asarray(Wv)]).astype(np.float32))
    wo_t = _round_fp32r(np.ascontiguousarray(np.asarray(Wo, np.float32).T)).reshape(ND, 128, D)
    bo_row = np.asarray(bo, np.float32).reshape(1, D)

    in_maps = []
    for c in range(NC_):
        w = W[:, 2 * c:2 * c + 2]                  # [3, 2, D, HD]
        w = np.ascontiguousarray(w.transpose(0, 2, 1, 3)).reshape(3, ND, 128, 128)
        in_maps.append({
            "embedded_t": emb_t,
            "w_qkv": w,
            "wo_t": wo_t,
            "bo_row": bo_row,
        })

    nc = _get_nc()
    res = run_bass_kernel_spmd(nc, in_maps, core_ids=list(range(NC_)),
                               trace=_trace)

    out = np.empty((B, S, D), np.float32)
    for c in range(NC_):
        s0 = (c % 2) * 1024
        out[c // 2, s0:s0 + 1024, :] = res.results[c]["out_shard"]
    if _trace:
        return out, res
    return out


# revision 19
# speedup vs baseline: 1.5132x; 1.0850x over previous
"""Multi-headed causal attention on 8 trn2 NeuronCores (Bass/Tile).

Sharding: tensor-parallel over heads — 2 heads per core, all 4 batches.
Per core:
  - Q^T/K^T/V^T projections with the 2 heads stacked on the partition axis
    (full 128-wide fp32r matmuls, contraction over D streamed from a
    host-side transposed copy of `embedded`).
  - scores computed transposed ([s_k, s_q] layout) in bf16 with K padded
    to 128 by zero rows (per-head K tiles carry zeros in the other head's
    partition range, so the stacked Q^T works as the moving operand for
    both heads and every matmul is the fast full-array shape).
  - both heads' score tiles land in one 2-bank PSUM tile; a single exp on
    ScalarE (no max subtraction needed: logits are ~N(0,1)); causal mask
    zeroes invalid entries via one affine_select on GpSimd.
  - AV matmul uses V padded to 128 columns: 64 V cols + a ones column
    (softmax denominators fall out as row 64) + zeros.
  - context is copied out of PSUM immediately (frees banks), normalized
    off the critical path, and staged for one 4MB AllToAll; row-sharded
    fp32r output projection + bias finishes it.
"""
import sys

sys.path.insert(0, "/opt/trn_rl_repo")

import numpy as np

import concourse.bass as bass
import concourse.tile as tile
from concourse import bacc, mybir
from concourse.bass_utils import run_bass_kernel_spmd

B, S, D, H, HD = 4, 2048, 1024, 16, 64
NC_ = 8          # cores
PH = 2           # heads per core
SC = 512         # s_q chunk (psum bank width in fp32)
NK = S // 128    # 16 s_k chunks of 128
ND = D // 128    # 8 contraction chunks of 128
F32 = mybir.dt.float32
F32R = mybir.dt.float32r
BF16 = mybir.dt.bfloat16
EXP = mybir.ActivationFunctionType.Exp
GE = mybir.AluOpType.is_ge


def build():
    nc = bacc.Bacc("TRN2", target_bir_lowering=False, debug=False, num_devices=NC_)

    emb_t = nc.dram_tensor("embedded_t", [B, D, S], F32R, kind="ExternalInput").ap()
    w_qkv = nc.dram_tensor("w_qkv", [3, ND, 128, 128], F32R, kind="ExternalInput").ap()
    wo_t = nc.dram_tensor("wo_t", [ND, 128, D], F32R, kind="ExternalInput").ap()
    bo_row = nc.dram_tensor("bo_row", [1, D], F32, kind="ExternalInput").ap()
    out_shard = nc.dram_tensor("out_shard", [1024, D], F32, kind="ExternalOutput").ap()

    with tile.TileContext(nc) as tc:
        _build_body(nc, tc, emb_t, w_qkv, wo_t, bo_row, out_shard)

    nc.compile()
    return nc


def _build_body(nc, tc, emb_t, w_qkv, wo_t, bo_row, out_shard):
    from contextlib import ExitStack

    ctx = ExitStack()
    with ctx:
        const = ctx.enter_context(tc.tile_pool(name="const", bufs=1))
        # "mm" slots are sized [128, 1024] (2 PSUM banks): 3x2 + ctx 2x1 = 8
        ps_mm = ctx.enter_context(tc.tile_pool(name="ps_mm", bufs=3, space="PSUM"))
        ps_ctx = ctx.enter_context(tc.tile_pool(name="ps_ctx", bufs=2, space="PSUM"))
        dram = ctx.enter_context(tc.tile_pool(name="dram", bufs=1, space="DRAM"))

        attn_ctx = ExitStack()
        etp = attn_ctx.enter_context(tc.tile_pool(name="etp", bufs=9))
        qtp = attn_ctx.enter_context(tc.tile_pool(name="qtp", bufs=2))
        ktp = attn_ctx.enter_context(tc.tile_pool(name="ktp", bufs=2))
        vtp = attn_ctx.enter_context(tc.tile_pool(name="vtp", bufs=1))
        vsb = attn_ctx.enter_context(tc.tile_pool(name="vsb", bufs=2))
        exp_p = attn_ctx.enter_context(tc.tile_pool(name="exp_p", bufs=5))
        cu_p = attn_ctx.enter_context(tc.tile_pool(name="cu_p", bufs=3))
        cn_p = attn_ctx.enter_context(tc.tile_pool(name="cn_p", bufs=3))
        rc_p = attn_ctx.enter_context(tc.tile_pool(name="rc_p", bufs=2))
        rb_p = attn_ctx.enter_context(tc.tile_pool(name="rb_p", bufs=2))

        # ---- prefetch batch 0 activations before anything else ----
        et_pre = {}
        for c in range(ND):
            t = etp.tile([128, S], F32R, tag="et", name=f"et0_{c}")
            nc.sync.dma_start(out=t[:], in_=emb_t[0, 128 * c:128 * (c + 1), :])
            et_pre[c] = t

        # ---- constants (wo_t loads happen in the output phase) ----
        # all 24 qkv weight chunks in one tile / one DMA
        wq_all = const.tile([128, 24, 128], F32R, tag="wq_all")
        nc.sync.dma_start(out=wq_all[:], in_=bass.AP(
            tensor=w_qkv.tensor, offset=0,
            ap=[[128, 128], [16384, 24], [1, 128]]))
        wq_sb = [[wq_all[:, 8 * p + c, :] for c in range(ND)] for p in range(3)]

        bo_sb = const.tile([1, D], F32, tag="bo1")
        nc.sync.dma_start(out=bo_sb[:], in_=bo_row[:])
        bo_b = const.tile([128, D], F32, tag="bob")
        nc.gpsimd.partition_broadcast(bo_b[:], bo_sb[:])

        ones_f32 = const.tile([128, 1], F32, tag="ones_f32")
        nc.vector.memset(ones_f32[:], 1.0)
        ones_r = const.tile([128, 1], BF16, tag="ones_r")
        nc.vector.tensor_copy(ones_r[:], ones_f32[:])

        ident = const.tile([128, 128], F32, tag="ident")
        nc.gpsimd.memset(ident[:], 1.0)
        nc.gpsimd.affine_select(out=ident[:], in_=ident[:], compare_op=GE,
                                fill=0.0, base=0, pattern=[[-1, 128]],
                                channel_multiplier=1)
        nc.gpsimd.affine_select(out=ident[:], in_=ident[:], compare_op=GE,
                                fill=0.0, base=0, pattern=[[1, 128]],
                                channel_multiplier=-1)

        a2a_in = [dram.tile([NC_, 128, 256], F32R, tag=f"a2a_in{q}",
                            name=f"a2a_in{q}") for q in range(4)]
        a2a_out = [dram.tile([NC_, 128, 256], F32R, tag=f"a2a_out{q}",
                             name=f"a2a_out{q}") for q in range(4)]

        def emit_a2a(q):
            nc.gpsimd.collective_compute(
                "AllToAll", mybir.AluOpType.bypass,
                replica_groups=[list(range(NC_))],
                ins=[a2a_in[q].opt()], outs=[a2a_out[q].opt()])

        # ---- per-batch: projections + attention ----
        for b in range(B):
            if b == 0:
                et = et_pre
            else:
                et = {}
                for c in range(ND):
                    t = etp.tile([128, S], F32R, tag="et", name=f"et{b}_{c}")
                    nc.sync.dma_start(
                        out=t[:], in_=emb_t[b, 128 * c:128 * (c + 1), :])
                    et[c] = t

            qt = qtp.tile([128, S], BF16, tag="qt")
            # per-head K^T padded to K=128 with zero rows for the other head
            kt0 = ktp.tile([128, S], BF16, tag="kt0")
            kt1 = ktp.tile([128, S], BF16, tag="kt1")
            nc.vector.memset(kt0[64:128, :], 0.0)
            nc.vector.memset(kt1[0:64, :], 0.0)
            vt = vtp.tile([128, S], F32, tag="vt")
            for j4 in range(4):          # s chunks of 512
                sl = slice(SC * j4, SC * (j4 + 1))
                for p in range(3):
                    ps = ps_mm.tile([128, SC], F32, tag="mm",
                                    name=f"pj{b}_{j4}_{p}")
                    for c in range(ND):
                        nc.tensor.matmul(
                            ps[:], lhsT=wq_sb[p][c],
                            rhs=et[c][:, sl],
                            start=(c == 0), stop=(c == ND - 1))
                    if p == 0:
                        nc.vector.tensor_copy(qt[:, sl], ps[:])
                    elif p == 1:
                        nc.vector.tensor_copy(kt0[0:64, sl], ps[0:64, :])
                        nc.vector.tensor_copy(kt1[64:128, sl], ps[64:128, :])
                    else:
                        nc.scalar.copy(vt[:, sl], ps[:])
            kts = [kt0, kt1]

            # V natural layout padded to 128 cols: V | ones | zeros
            v01 = [vsb.tile([128, NK, 128], BF16, tag=f"v{h}", name=f"v{h}")
                   for h in range(PH)]
            for h in range(PH):
                nc.vector.memset(v01[h][:, :, 65:128], 0.0)
            for sk in range(NK):
                pt = ps_mm.tile([128, 128], F32, tag="mm", name=f"tr{b}_{sk}")
                nc.tensor.transpose(pt[:], vt[:, 128 * sk:128 * (sk + 1)], ident[:])
                for h in range(PH):
                    nc.vector.tensor_copy(v01[h][:, sk, 0:64],
                                          pt[:, 64 * h:64 * (h + 1)])
                    nc.vector.tensor_copy(v01[h][:, sk, 64:65], ones_r[:])

            # attention, one s_q chunk of 512 at a time
            # even j first: owner-block cols [0:512) complete after the
            # last batch's j=2, letting A2A #0/#1 overlap the odd-j work
            for j in (0, 2, 1, 3):
                mtop = 4 * j + 4
                ctx_ps = [ps_ctx.tile([128, SC], F32, tag="ctx",
                                      name=f"ctx{b}_{j}_{h}")
                          for h in range(PH)]
                PIPE = 2   # scores run this many m-iterations ahead of AV
                exq = []   # (m, ex) awaiting AV

                def emit_scores(m):
                    psc = ps_mm.tile([128, 2 * SC], F32, tag="mm",
                                     name=f"sc{b}_{j}_{m}")
                    for h in range(PH):
                        nc.tensor.matmul(
                            psc[:, SC * h:SC * (h + 1)],
                            lhsT=kts[h][:, 128 * m:128 * (m + 1)],
                            rhs=qt[:, SC * j:SC * (j + 1)],
                            start=True, stop=True)
                    ex = exp_p.tile([128, 2 * SC], BF16, tag="ex",
                                    name=f"ex{b}_{j}_{m}")
                    nc.scalar.activation(out=ex[:], in_=psc[:], func=EXP,
                                         scale=0.125)
                    if m >= 4 * j:  # diagonal tile: zero k>q entries
                        nc.gpsimd.affine_select(
                            out=ex[:], in_=ex[:], compare_op=GE, fill=0.0,
                            base=SC * j - 128 * m, pattern=[[0, 2], [1, SC]],
                            channel_multiplier=-1)
                    exq.append((m, ex))

                def emit_av():
                    m_av, ex = exq.pop(0)
                    for h in range(PH):
                        nc.tensor.matmul(
                            ctx_ps[h][:], lhsT=v01[h][:, m_av, :],
                            rhs=ex[:, SC * h:SC * (h + 1)],
                            start=(m_av == 0), stop=(m_av == mtop - 1))

                for m in range(mtop):
                    emit_scores(m)
                    if len(exq) > PIPE:
                        emit_av()
                while exq:
                    emit_av()

                # free PSUM fast, then normalize off the critical path
                # q-quarter of the owner block: even j -> q 0/1, odd -> 2/3
                o, q0 = 2 * b + j // 2, 2 * (j % 2)
                for h in range(PH):
                    cu = cu_p.tile([65, SC], F32, tag="cu",
                                   name=f"cu{b}_{j}_{h}")
                    nc.scalar.copy(cu[:], ctx_ps[h][0:65, :])
                    rc = rc_p.tile([1, SC], F32, tag="rc")
                    nc.vector.reciprocal(rc[:], cu[64:65, :])
                    rb = rb_p.tile([64, SC], F32, tag="rb")
                    nc.gpsimd.partition_broadcast(rb[:], rc[:])
                    cn = cn_p.tile([64, SC], F32R, tag="cn")
                    nc.vector.tensor_mul(cn[:], cu[0:64, :], rb[:])
                    hr = slice(64 * h, 64 * (h + 1))
                    nc.sync.dma_start(out=a2a_in[q0][o, hr, :],
                                      in_=cn[:, 0:256])
                    nc.sync.dma_start(out=a2a_in[q0 + 1][o, hr, :],
                                      in_=cn[:, 256:512])
                if b == B - 1 and j == 2:
                    emit_a2a(0)
                    emit_a2a(1)

        # ---- remaining all-to-alls + row-sharded output projection ----
        attn_ctx.close()
        emit_a2a(2)
        emit_a2a(3)

        cat_p = ctx.enter_context(tc.tile_pool(name="cat_p", bufs=32))
        ob_p = ctx.enter_context(tc.tile_pool(name="ob_p", bufs=3))
        wot_sb = [const.tile([128, D], F32R, tag=f"wo{c}", name=f"wo{c}")
                  for c in range(ND)]
        for c in range(ND):
            nc.sync.dma_start(out=wot_sb[c][:], in_=wo_t[c])
        for q in range(4):
            cats = []
            for r in range(NC_):
                ct = cat_p.tile([128, 256], F32R, tag=f"cat{q}",
                                name=f"cat{q}_{r}")
                nc.sync.dma_start(out=ct[:], in_=a2a_out[q][r])
                cats.append(ct)
            for sq in (2 * q, 2 * q + 1):
                lo = 128 * (sq % 2)
                for n in range(2):
                    po = ps_mm.tile([128, SC], F32, tag="mm",
                                    name=f"po{sq}_{n}")
                    for kp in range(ND):
                        nc.tensor.matmul(
                            po[:], lhsT=cats[kp][:, lo:lo + 128],
                            rhs=wot_sb[kp][:, SC * n:SC * (n + 1)],
                            start=(kp == 0), stop=(kp == ND - 1))
                    ob = ob_p.tile([128, SC], F32, tag="ob")
                    nc.vector.tensor_add(ob[:], po[:],
                                         bo_b[:, SC * n:SC * (n + 1)])
                    nc.sync.dma_start(
                        out=out_shard[128 * sq:128 * (sq + 1),
                                      SC * n:SC * (n + 1)],
                        in_=ob[:])


_NC_CACHE = None


def _get_nc():
    global _NC_CACHE
    if _NC_CACHE is None:
        _NC_CACHE = build()
    return _NC_CACHE


def _round_fp32r(x):
    """Round fp32 to fp32r (11-bit mantissa, RNE) — what the PE expects."""
    u = np.ascontiguousarray(x, np.float32).view(np.uint32)
    r = (u + np.uint32(0x7FF) + ((u >> np.uint32(12)) & np.uint32(1))) & np.uint32(0xFFFFF000)
    return r.view(np.float32)


def kernel(embedded, Wq, Wk, Wv, Wo, bo, _trace=False):
    embedded = np.ascontiguousarray(np.asarray(embedded, np.float32))
    emb_t = _round_fp32r(np.ascontiguousarray(embedded.transpose(0, 2, 1)))
    W = _round_fp32r(np.stack([np.asarray(Wq), np.asarray(Wk), np.asarray(Wv)]).astype(np.float32))
    wo_t = _round_fp32r(np.ascontiguousarray(np.asarray(Wo, np.float32).T)).reshape(ND, 128, D)
    bo_row = np.asarray(bo, np.float32).reshape(1, D)

    in_maps = []
    for c in range(NC_):
        w = W[:, 2 * c:2 * c + 2]                  # [3, 2, D, HD]
        w = np.ascontiguousarray(w.transpose(0, 2, 1, 3)).reshape(3, ND, 128, 128)
        in_maps.append({
            "embedded_t": emb_t,
            "w_qkv": w,
            "wo_t": wo_t,
            "bo_row": bo_row,
        })

    nc = _get_nc()
    res = run_bass_kernel_spmd(nc, in_maps, core_ids=list(range(NC_)),
                               trace=_trace)

    out = np.empty((B, S, D), np.float32)
    for c in range(NC_):
        s0 = (c % 2) * 1024
        out[c // 2, s0:s0 + 1024, :] = res.results[c]["out_shard"]
    if _trace:
        return out, res
    return out
